# revision 20
# baseline (speedup 1.0000x reference)
"""Paged KV-cache scatter (vLLM basic_cache) on 8 Trainium2 NeuronCores.

Strategy: shard the flat slot space (num_blocks dim) across the 8 cores;
route tokens host-side to the core owning their slot.  On each core the
scatter is reformulated as a gather: for each of the core's 16384 output
slot-rows, the source row is either the old kv row (survivor) or the new
token row, both living in one concatenated DRAM source tensor.  The device
kernel is a pipeline of GPSIMD dma_gather instructions (one per 2048 rows)
and contiguous HWDGE stores, writing every output byte exactly once:
~128MB of HBM traffic per core instead of 192MB for copy-then-scatter.
"""

import sys

import numpy as np

sys.path.insert(0, "/opt/trn_rl_repo")

NUM_BLOCKS = 8192
BLOCK_SIZE = 16
NUM_HEADS = 8
HEAD_SIZE = 128
NUM_TOKENS = 65536
ROW = NUM_HEADS * HEAD_SIZE  # 1024 f32 = 4096 B per token/slot row
N_CORES = 8
SLOTS_TOTAL = NUM_BLOCKS * BLOCK_SIZE  # 131072
SLOTS_PER_CORE = SLOTS_TOTAL // N_CORES  # 16384

P = 128  # SBUF partitions
NI = SLOTS_PER_CORE // P  # gather/store instruction pairs per core (128)

TRACE = False  # test harness sets True to collect a HW profile
LAST_RESULT = None  # BassKernelResults of the last run (for test harness)
DATA_BUFS = 12
TOKEN_SORT = False  # sort routed tokens by slot (no measured effect)
STORE_SPLIT = False  # alternate stores between sync (SP) and scalar (ACT)

_prog_cache = {}


def _build_program(nsrc):
    import concourse.bacc as bacc
    import concourse.bass as bass
    import concourse.tile as tile
    from concourse import mybir

    nc = bacc.Bacc()
    src = nc.declare_dram_parameter(
        "src", [nsrc, ROW], mybir.dt.float32, isOutput=False
    )
    gidx = nc.declare_dram_parameter(
        "gidx", [P, NI], mybir.dt.int32, isOutput=False
    )
    out = nc.declare_dram_parameter(
        "out", [SLOTS_PER_CORE, ROW], mybir.dt.float32, isOutput=True
    )

    with tile.TileContext(nc) as tc:
        with (
            tc.tile_pool(name="idx", bufs=1) as idx_pool,
            tc.tile_pool(name="data", bufs=DATA_BUFS) as data_pool,
        ):
            gidx_sb = idx_pool.tile([P, NI], mybir.dt.int32)
            nc.gpsimd.dma_start(out=gidx_sb[:], in_=gidx[:])
            for i in range(NI):
                t = data_pool.tile([P, ROW], mybir.dt.float32)
                nc.gpsimd.indirect_dma_start(
                    out=t[:],
                    out_offset=None,
                    in_=src[:, :],
                    in_offset=bass.IndirectOffsetOnAxis(
                        ap=gidx_sb[:, i : i + 1], axis=0
                    ),
                )
                st_eng = nc.scalar if (STORE_SPLIT and i % 2) else nc.sync
                st_eng.dma_start(out=out[i * P : (i + 1) * P, :], in_=t[:])
    return nc


def prepare(to_cache, kv_cache, slot_mapping):
    """Host-side routing: build per-core in_maps and the padded src size."""
    to_flat = np.ascontiguousarray(np.asarray(to_cache)).reshape(NUM_TOKENS, ROW)
    kv_flat = np.ascontiguousarray(np.asarray(kv_cache)).reshape(SLOTS_TOTAL, ROW)
    slots = np.asarray(slot_mapping).astype(np.int64)

    core_of = slots // SLOTS_PER_CORE
    sels = [np.nonzero(core_of == d)[0] for d in range(N_CORES)]
    max_tok = max(len(s) for s in sels)
    # round up so small input variations reuse the compiled program
    nsrc = SLOTS_PER_CORE + ((max_tok + 511) // 512) * 512

    in_maps = []
    for d in range(N_CORES):
        sel = sels[d]
        lsl = slots[sel] - d * SLOTS_PER_CORE
        if TOKEN_SORT:
            order = np.argsort(lsl, kind="stable")
            sel = sel[order]
            lsl = lsl[order]
        gidx = np.arange(SLOTS_PER_CORE, dtype=np.int32)
        gidx[lsl] = SLOTS_PER_CORE + np.arange(len(sel), dtype=np.int32)
        src = np.empty((nsrc, ROW), np.float32)
        src[:SLOTS_PER_CORE] = kv_flat[d * SLOTS_PER_CORE : (d + 1) * SLOTS_PER_CORE]
        src[SLOTS_PER_CORE : SLOTS_PER_CORE + len(sel)] = to_flat[sel]
        # instruction i gathers output rows [i*P, (i+1)*P): partition p's
        # source row sits at SBUF [p, i]
        gidx_re = np.ascontiguousarray(gidx.reshape(NI, P).T)
        in_maps.append({"src": src, "gidx": gidx_re})
    return in_maps, nsrc


def get_program(nsrc):
    key = (nsrc, DATA_BUFS, STORE_SPLIT)
    if key not in _prog_cache:
        nc = _build_program(nsrc)
        nc.finalize()  # Bacc.compile(): reg alloc + sync-wait legalization
        _prog_cache[key] = nc
    return _prog_cache[key]


def kernel(to_cache, kv_cache, slot_mapping):
    global LAST_RESULT
    from concourse.bass_utils import run_bass_kernel_spmd

    in_maps, nsrc = prepare(to_cache, kv_cache, slot_mapping)
    nc = get_program(nsrc)

    res = run_bass_kernel_spmd(nc, in_maps, list(range(N_CORES)), trace=TRACE)
    LAST_RESULT = res
    full = np.concatenate(
        [res.results[d]["out"] for d in range(N_CORES)], axis=0
    )
    return full.reshape(NUM_BLOCKS, BLOCK_SIZE, NUM_HEADS, HEAD_SIZE)


# revision 25
# speedup vs baseline: 1.2917x; 1.2917x over previous
"""Paged KV-cache scatter (vLLM basic_cache) on 8 Trainium2 NeuronCores.

Strategy: shard the flat slot space (num_blocks dim) across the 8 cores;
route tokens host-side to the core owning their slot.  On each core the
scatter is reformulated as a gather: for each of the core's 16384 output
slot-rows, the source row is either the old kv row (survivor) or the new
token row, both living in one concatenated DRAM source tensor.  The device
kernel is a pipeline of GPSIMD dma_gather instructions (one per 2048 rows)
and contiguous HWDGE stores, writing every output byte exactly once:
~128MB of HBM traffic per core instead of 192MB for copy-then-scatter.
"""

import sys

import numpy as np

sys.path.insert(0, "/opt/trn_rl_repo")

NUM_BLOCKS = 8192
BLOCK_SIZE = 16
NUM_HEADS = 8
HEAD_SIZE = 128
NUM_TOKENS = 65536
ROW = NUM_HEADS * HEAD_SIZE  # 1024 f32 = 4096 B per token/slot row
N_CORES = 8
SLOTS_TOTAL = NUM_BLOCKS * BLOCK_SIZE  # 131072
SLOTS_PER_CORE = SLOTS_TOTAL // N_CORES  # 16384

P = 128  # SBUF partitions
NI = SLOTS_PER_CORE // P  # gather/store instruction pairs per core (128)

TRACE = False  # test harness sets True to collect a HW profile
LAST_RESULT = None  # BassKernelResults of the last run (for test harness)
DATA_BUFS = 12
TOKEN_SORT = False  # sort routed tokens by slot (no measured effect)
STORE_SPLIT = False  # alternate stores between sync (SP) and scalar (ACT)
PAIR = 1  # gathers per tile; one store covers PAIR gathers

_prog_cache = {}


def _build_program(nsrc):
    import concourse.bacc as bacc
    import concourse.bass as bass
    import concourse.tile as tile
    from concourse import mybir

    nc = bacc.Bacc()
    src = nc.declare_dram_parameter(
        "src", [nsrc, ROW], mybir.dt.float32, isOutput=False
    )
    gidx = nc.declare_dram_parameter(
        "gidx", [P, NI], mybir.dt.int32, isOutput=False
    )
    out = nc.declare_dram_parameter(
        "out", [SLOTS_PER_CORE, ROW], mybir.dt.float32, isOutput=True
    )

    with tile.TileContext(nc) as tc:
        with (
            tc.tile_pool(name="idx", bufs=1) as idx_pool,
            tc.tile_pool(name="data", bufs=DATA_BUFS) as data_pool,
        ):
            gidx_sb = idx_pool.tile([P, NI], mybir.dt.int32)
            nc.gpsimd.dma_start(out=gidx_sb[:], in_=gidx[:])
            for i0 in range(0, NI, PAIR):
                t = data_pool.tile([P, PAIR * ROW], mybir.dt.float32)
                for j in range(PAIR):
                    i = i0 + j
                    nc.gpsimd.indirect_dma_start(
                        out=t[:, j * ROW : (j + 1) * ROW],
                        out_offset=None,
                        in_=src[:, :],
                        in_offset=bass.IndirectOffsetOnAxis(
                            ap=gidx_sb[:, i : i + 1], axis=0
                        ),
                    )
                st_eng = nc.scalar if (STORE_SPLIT and (i0 // PAIR) % 2) else nc.sync
                # SBUF (p, j*ROW:(j+1)*ROW) holds output row i0*P + p*PAIR + j
                # (host lays gidx out accordingly), so each partition's PAIR
                # rows are CONSECUTIVE in DRAM -> PAIR*4KB descriptors
                st_eng.dma_start(
                    out=out[i0 * P : (i0 + PAIR) * P, :].rearrange(
                        "(p j) d -> p (j d)", p=P
                    ),
                    in_=t[:],
                )
    return nc


def prepare(to_cache, kv_cache, slot_mapping):
    """Host-side routing: build per-core in_maps and the padded src size."""
    to_flat = np.ascontiguousarray(np.asarray(to_cache)).reshape(NUM_TOKENS, ROW)
    kv_flat = np.ascontiguousarray(np.asarray(kv_cache)).reshape(SLOTS_TOTAL, ROW)
    slots = np.asarray(slot_mapping).astype(np.int64)

    core_of = slots // SLOTS_PER_CORE
    sels = [np.nonzero(core_of == d)[0] for d in range(N_CORES)]
    max_tok = max(len(s) for s in sels)
    # round up so small input variations reuse the compiled program
    nsrc = SLOTS_PER_CORE + ((max_tok + 511) // 512) * 512

    in_maps = []
    for d in range(N_CORES):
        sel = sels[d]
        lsl = slots[sel] - d * SLOTS_PER_CORE
        if TOKEN_SORT:
            order = np.argsort(lsl, kind="stable")
            sel = sel[order]
            lsl = lsl[order]
        gidx = np.arange(SLOTS_PER_CORE, dtype=np.int32)
        gidx[lsl] = SLOTS_PER_CORE + np.arange(len(sel), dtype=np.int32)
        src = np.empty((nsrc, ROW), np.float32)
        src[:SLOTS_PER_CORE] = kv_flat[d * SLOTS_PER_CORE : (d + 1) * SLOTS_PER_CORE]
        src[SLOTS_PER_CORE : SLOTS_PER_CORE + len(sel)] = to_flat[sel]
        # instruction block b covers output rows [b*P*PAIR, (b+1)*P*PAIR);
        # within it, gather j / partition p handles row b*P*PAIR + p*PAIR + j
        # and reads its source row from SBUF [p, b*PAIR + j]
        nb = NI // PAIR
        gidx_re = np.ascontiguousarray(
            gidx.reshape(nb, P, PAIR).transpose(1, 0, 2).reshape(P, NI)
        )
        in_maps.append({"src": src, "gidx": gidx_re})
    return in_maps, nsrc


def get_program(nsrc):
    key = (nsrc, DATA_BUFS, STORE_SPLIT, PAIR)
    if key not in _prog_cache:
        nc = _build_program(nsrc)
        nc.finalize()  # Bacc.compile(): reg alloc + sync-wait legalization
        _prog_cache[key] = nc
    return _prog_cache[key]


def kernel(to_cache, kv_cache, slot_mapping):
    global LAST_RESULT
    from concourse.bass_utils import run_bass_kernel_spmd

    in_maps, nsrc = prepare(to_cache, kv_cache, slot_mapping)
    nc = get_program(nsrc)

    res = run_bass_kernel_spmd(nc, in_maps, list(range(N_CORES)), trace=TRACE)
    LAST_RESULT = res
    full = np.concatenate(
        [res.results[d]["out"] for d in range(N_CORES)], axis=0
    )
    return full.reshape(NUM_BLOCKS, BLOCK_SIZE, NUM_HEADS, HEAD_SIZE)
